# revision 5
# baseline (speedup 1.0000x reference)
"""Trainium2 Bass kernel for nn_Net_76562087018570.

Computation (reference): per-column MinMax scale of a (4096, 8192) f32 matrix,
10 iterations of arr = arr*(1 - (arr - rowmean(arr))) (+0.001 on iter 0),
then inverse transform.  Rows sharded 8 ways (512 rows/core).

v2 design:
- All data f16 in SBUF.  Inflow f32 quarters land in 2 rotating staging
  slots, ACT casts to the persistent f16 tile D while DVE computes the
  per-column min/max combine tree in f16 (2x mode), chasing the input DMA.
- Column min/max: pairwise f16 max/min combines (last step emits f32),
  gpsimd partition_all_reduce for the max side, PE transposes + DVE min
  reduces for the min side, one packed AllReduce(max) on [max | -min].
  A tiny warmup AllReduce + warmup PAR at t=0 pay the collective barrier
  and gpsimd library load during the load phase (saves ~100us).
- Iteration state: ACT slice (A cols) carries s_k = (arr_k - C_k)^2 and
  uses one Square activation per pass (bias=G, scale=-1, fused row-sum).
  DVE slice (V cols) carries y_k = s_k + lambda_k, for which the update
  collapses to ONE scalar_tensor_tensor per pass: y' = (y - 2g)*y at f16
  2x mode, with fused row-sum.  Per-row scalar chains (10 tiny DVE ops
  per group-pass) track h/C/G/lambda.
- Startup ((a-mn)*rinv, f16 2x) and final (mn + safe*(gam - state), f16 2x)
  are two DVE ops each; ACT converts the f16 result to f32 into the dead
  f32 staging slots for DMA out.  Broadcast vectors (mn, rinv, safe) are
  f16.  Two tile groups (2 tiles each) pipeline passes/chains/finals.
"""

import os
import numpy as np

R = 512          # rows per core
N = 8192         # columns
NT = 4           # (128,N) row tiles per core
NQ = 4           # column quarters
QW = N // NQ     # 2048
NCORES = 8
NPASS = 10
A = 4032         # ACT-slice columns (s-state)
V = N - A        # DVE-slice columns (y-state), 4160

_cache = {}
LAST_RESULT = None


def _build():
    import concourse.bacc as bacc
    import concourse.tile as tile
    from concourse import mybir, masks, bass_isa

    f32 = mybir.dt.float32
    f16 = mybir.dt.float16
    Al = mybir.AluOpType
    AF = mybir.ActivationFunctionType
    AX = mybir.AxisListType

    nc = bacc.Bacc(trn_type="TRN2", num_devices=NCORES)
    xs = nc.dram_tensor("xs", [R, N], f32, kind="ExternalInput")
    out = nc.dram_tensor("out", [R, N], f32, kind="ExternalOutput")
    xv = xs.ap().rearrange("(t p) n -> t p n", p=128)
    ov = out.ap().rearrange("(t p) n -> t p n", p=128)

    with tile.TileContext(nc) as tc:
        with tc.tile_pool(name="rot", bufs=1) as rot, \
             tc.tile_pool(name="data", bufs=1) as data, \
             tc.tile_pool(name="mmq", bufs=1) as mmq, \
             tc.tile_pool(name="small", bufs=1) as small, \
             tc.tile_pool(name="psumT", bufs=2, space="PSUM") as psumT, \
             tc.tile_pool(name="dram", bufs=1, space="DRAM") as dram:

            ident = small.tile([128, 128], f32)
            masks.make_identity(nc, ident[:])

            # ---- warmups: collective barrier + ring setup, gpsimd PAR
            # library load, ACT Square table load -- all during the input DMA
            wc_in = dram.tile([1, 8], f32)
            wc_out = dram.tile([1, 8], f32, addr_space="Shared")
            nc.sync.dma_start(wc_in[:], ident[0:1, 0:8])
            nc.gpsimd.collective_compute(
                "AllReduce", Al.max,
                replica_groups=[list(range(NCORES))],
                ins=[wc_in[:]], outs=[wc_out[:]],
            )
            wp = small.tile([128, 8], f32)
            nc.gpsimd.partition_all_reduce(
                wp[:], ident[:, 0:8], channels=128,
                reduce_op=bass_isa.ReduceOp.max)
            wsq = small.tile([128, 8], f32)
            nc.scalar.activation(wsq[:], ident[:, 0:8], AF.Square)

            # ---- persistent f16 data: D[:, t*N : (t+1)*N] = tile t ----
            D = data.tile([128, NT * N], f16, name="D")
            Dv3 = D[:].rearrange("p (t n) -> p t n", t=NT)

            def dseg(t, lo, hi):
                return D[:, t * N + lo: t * N + hi]

            # ---- phase 1: load quarters into rotating f32 slots, cast to
            # f16 (ACT), combine min/max (DVE f16), PAR (gpsimd) + PE/DVE
            # min reduction, pack collective input ----
            rsl = [rot.tile([128, N], f32, name=f"rs{b}") for b in range(2)]
            cmax2 = mmq.tile([128, QW], f32, name="cmax2")
            cmin2 = mmq.tile([128, QW], f32, name="cmin2")
            rmin = small.tile([128, 64], f32)
            cc_in = dram.tile([2, N], f32)
            cc_out = dram.tile([2, N], f32, addr_space="Shared")

            for j in range(NQ):
                b = j % 2
                qlo = j * QW
                for t in range(NT):
                    nc.sync.dma_start(rsl[b][:, t * QW:(t + 1) * QW],
                                      xv[t][:, qlo:qlo + QW])
                # cast the whole quarter (all 4 tiles) in one ACT op
                nc.scalar.copy(
                    Dv3[:, :, qlo:qlo + QW],
                    rsl[b][:].rearrange("p (t c) -> p t c", t=NT))
                # f16 combine trees; last step emits f32
                tmx = mmq.tile([128, QW], f16, name="tmx")
                nc.vector.tensor_tensor(tmx[:], dseg(0, qlo, qlo + QW),
                                        dseg(1, qlo, qlo + QW), op=Al.max)
                nc.vector.tensor_tensor(tmx[:], tmx[:],
                                        dseg(2, qlo, qlo + QW), op=Al.max)
                nc.vector.tensor_tensor(cmax2[:], tmx[:],
                                        dseg(3, qlo, qlo + QW), op=Al.max)
                tmn = mmq.tile([128, QW], f16, name="tmn")
                nc.vector.tensor_tensor(tmn[:], dseg(0, qlo, qlo + QW),
                                        dseg(1, qlo, qlo + QW), op=Al.min)
                nc.vector.tensor_tensor(tmn[:], tmn[:],
                                        dseg(2, qlo, qlo + QW), op=Al.min)
                nc.vector.tensor_tensor(cmin2[:], tmn[:],
                                        dseg(3, qlo, qlo + QW), op=Al.min)
                # min side: PE transposes + DVE min reduce ->
                # rmin[p, j*16+cb] = min of col j*2048 + cb*128 + p
                for g in range(2):
                    pt = psumT.tile([128, 1024], f32, name="pt", tag="pst")
                    for b8 in range(8):
                        cb = g * 8 + b8
                        nc.tensor.transpose(
                            pt[:, b8 * 128:(b8 + 1) * 128],
                            cmin2[:, cb * 128:(cb + 1) * 128],
                            ident[:])
                    nc.vector.tensor_reduce(
                        out=rmin[:, j * 16 + g * 8:j * 16 + g * 8 + 8],
                        in_=pt[:].rearrange("p (c x) -> p c x", c=8),
                        axis=AX.X, op=Al.min)
                # max side: PAR into cmin2 (dead after transposes)
                nc.gpsimd.partition_all_reduce(
                    cmin2[:], cmax2[:], channels=128,
                    reduce_op=bass_isa.ReduceOp.max)
                nc.sync.dma_start(cc_in[0:1, qlo:qlo + QW], cmin2[0:1, :])

            nrmin = small.tile([128, 64], f32)
            nc.vector.tensor_scalar(out=nrmin[:], in0=rmin[:], scalar1=-1.0,
                                    scalar2=None, op0=Al.mult)
            nc.sync.dma_start(
                cc_in[1:2, :].rearrange("o (p f) -> (o p) f", p=128), nrmin[:])

            # ---- AllReduce(max) on [gmax | -min] ----
            nc.gpsimd.collective_compute(
                "AllReduce", Al.max,
                replica_groups=[list(range(NCORES))],
                ins=[cc_in[:]], outs=[cc_out[:]],
            )

            # ---- post-collective scalar math in partition-major (128,64) ----
            gmr = small.tile([64, 128], f32)
            nc.sync.dma_start(
                gmr[:], cc_out[0:1, :].rearrange("o (f p) -> (o f) p", p=128))
            tg = psumT.tile([128, 64], f32, name="tg", tag="pst")
            nc.tensor.transpose(tg[:], gmr[:], ident[0:64, 0:64])
            gmaxP = small.tile([128, 64], f32)
            nc.scalar.copy(gmaxP[:], tg[:])
            nminP = small.tile([128, 64], f32)
            nc.sync.dma_start(
                nminP[:],
                cc_out[1:2, :].rearrange("o (p f) -> (o p) f", p=128))

            rng = small.tile([128, 64], f32)
            nc.vector.tensor_tensor(rng[:], gmaxP[:], nminP[:], op=Al.add)
            eq0 = small.tile([128, 64], f32)
            nc.vector.tensor_scalar(out=eq0[:], in0=rng[:], scalar1=0.0,
                                    scalar2=None, op0=Al.is_equal)
            safe = small.tile([128, 64], f32)
            nc.vector.tensor_tensor(safe[:], rng[:], eq0[:], op=Al.add)
            rinv = small.tile([128, 64], f32)
            nc.vector.reciprocal(rinv[:], safe[:])
            minP = small.tile([128, 64], f32)
            nc.vector.tensor_scalar(out=minP[:], in0=nminP[:], scalar1=-1.0,
                                    scalar2=None, op0=Al.mult)

            # relayout to natural rows via pack + PE transposes, f16 out
            packa = small.tile([128, 128], f32)
            nc.vector.tensor_copy(packa[:, 0:64], minP[:])
            nc.vector.tensor_copy(packa[:, 64:128], rinv[:])
            ta = psumT.tile([128, 128], f32, name="ta", tag="pst")
            nc.tensor.transpose(ta[:], packa[:], ident[:])
            tas = small.tile([128, 128], f16)
            nc.scalar.copy(tas[:], ta[:])
            tb = psumT.tile([64, 128], f32, name="tb", tag="pst")
            nc.tensor.transpose(tb[:], safe[:], ident[:])
            tbs = small.tile([64, 128], f16)
            nc.scalar.copy(tbs[:], tb[:])

            mn_d = dram.tile([1, N], f16)
            rinv_d = dram.tile([1, N], f16)
            safe_d = dram.tile([1, N], f16)
            nc.sync.dma_start(
                mn_d[:].rearrange("o (f p) -> (o f) p", p=128), tas[0:64, :])
            nc.sync.dma_start(
                rinv_d[:].rearrange("o (f p) -> (o f) p", p=128),
                tas[64:128, :])
            nc.sync.dma_start(
                safe_d[:].rearrange("o (f p) -> (o f) p", p=128), tbs[:])

            # full-width f16 broadcasts
            mnb = data.tile([128, N], f16, name="mnb")
            rb = data.tile([128, N], f16, name="rb")
            nc.sync.dma_start(mnb[:], mn_d[0:1, :].to_broadcast((128, N)))
            nc.sync.dma_start(rb[:], rinv_d[0:1, :].to_broadcast((128, N)))

            # ---- startup: arr0 = (a' - mn)*rinv in place, f16 2x ----
            sarr = small.tile([128, NT], f32)

            def startup(t):
                Dt = dseg(t, 0, N)
                nc.vector.tensor_tensor(Dt, Dt, mnb[:], op=Al.subtract)
                nc.vector.scalar_tensor_tensor(
                    out=Dt, in0=Dt, scalar=0.0, in1=rb[:],
                    op0=Al.bypass, op1=Al.mult,
                    accum_out=sarr[:, t:t + 1])

            def bc_safeb():
                sb = data.tile([128, N], f16, name="rb")
                nc.sync.dma_start(sb[:], safe_d[0:1, :].to_broadcast((128, N)))
                return sb

            # ---- per-group (2 tiles) scalar chains, (128,2) f32 tiles ----
            G = [dict() for _ in range(2)]

            _ntc = [0]

            def nt_(shape=(128, 2)):
                _ntc[0] += 1
                return small.tile(list(shape), f32, name=f"ch{_ntc[0]}")

            def ginit(g):
                st = G[g]
                h0 = nt_()
                nc.vector.tensor_scalar(out=h0[:], in0=sarr[:, 2 * g:2 * g + 2],
                                        scalar1=1.0 / N, scalar2=None,
                                        op0=Al.mult)
                C = nt_()
                nc.vector.tensor_scalar(out=C[:], in0=h0[:], scalar1=0.5,
                                        scalar2=0.5, op0=Al.mult, op1=Al.add)
                qr = nt_()
                nc.vector.tensor_tensor(qr[:], C[:], C[:], op=Al.mult)
                q = nt_()
                nc.vector.tensor_scalar(out=q[:], in0=qr[:], scalar1=0.001,
                                        scalar2=None, op0=Al.add)
                g2 = nt_()
                nc.vector.tensor_scalar(out=g2[:], in0=C[:], scalar1=2.0,
                                        scalar2=None, op0=Al.mult)
                Lm = nt_()
                nc.vector.scalar_tensor_tensor(
                    out=Lm[:], in0=C[:], scalar=-2.0, in1=C[:],
                    op0=Al.mult, op1=Al.mult)
                mu = nt_()
                nc.vector.tensor_scalar(out=mu[:], in0=Lm[:], scalar1=V / 2.0,
                                        scalar2=None, op0=Al.mult)
                st.update(q=q, Lm=Lm, mu=mu, bias=C, g2=g2)

            def gpass(g, k, last=False):
                st = G[g]
                accA = nt_() if not last else None
                accD = nt_() if not last else None
                st["accA"], st["accD"] = accA, accD
                for i, t in enumerate((2 * g, 2 * g + 1)):
                    nc.scalar.activation(
                        dseg(t, 0, A), dseg(t, 0, A), AF.Square,
                        bias=st["bias"][:, i:i + 1], scale=-1.0,
                        accum_out=(None if last else accA[:, i:i + 1]))
                    DtV = dseg(t, A, N)
                    nc.vector.scalar_tensor_tensor(
                        out=DtV, in0=DtV, scalar=st["g2"][:, i:i + 1],
                        in1=DtV, op0=Al.subtract, op1=Al.mult,
                        accum_out=(None if last else accD[:, i:i + 1]))

            def gchain(g):
                st = G[g]
                u = nt_()
                nc.vector.tensor_tensor(u[:], st["accA"][:], st["accD"][:],
                                        op=Al.add)
                S = nt_()
                nc.vector.tensor_tensor(S[:], u[:], st["mu"][:],
                                        op=Al.subtract)
                t1 = nt_()
                nc.vector.tensor_scalar(out=t1[:], in0=S[:],
                                        scalar1=-1.0 / N, scalar2=None,
                                        op0=Al.mult)
                h = nt_()
                nc.vector.tensor_tensor(h[:], t1[:], st["q"][:], op=Al.add)
                C2 = nt_()
                nc.vector.tensor_scalar(out=C2[:], in0=h[:], scalar1=0.5,
                                        scalar2=0.5, op0=Al.mult, op1=Al.add)
                Gb = nt_()
                nc.vector.tensor_tensor(Gb[:], st["q"][:], C2[:],
                                        op=Al.subtract)
                q2 = nt_()
                nc.vector.tensor_tensor(q2[:], C2[:], C2[:], op=Al.mult)
                g2 = nt_()
                nc.vector.scalar_tensor_tensor(
                    out=g2[:], in0=Gb[:], scalar=2.0, in1=st["Lm"][:],
                    op0=Al.mult, op1=Al.add)
                Lm2 = nt_()
                nc.vector.scalar_tensor_tensor(
                    out=Lm2[:], in0=g2[:], scalar=-0.5, in1=g2[:],
                    op0=Al.mult, op1=Al.mult)
                mu2 = nt_()
                nc.vector.tensor_scalar(out=mu2[:], in0=Lm2[:],
                                        scalar1=V / 2.0, scalar2=None,
                                        op0=Al.mult)
                st.update(q=q2, Lm=Lm2, mu=mu2, bias=Gb, g2=g2)

            def gfinal_scalars(g):
                st = G[g]
                gamV = nt_()
                nc.vector.scalar_tensor_tensor(
                    out=gamV[:], in0=st["Lm"][:], scalar=0.5, in1=st["q"][:],
                    op0=Al.mult, op1=Al.add)
                st["gamV"] = gamV      # = q9 + lambda9 for the y slice
                st["gamA"] = st["q"]   # = q9 = C9^2 for the s slice

            def final(t, safeb):
                g, i = t // 2, t % 2
                st = G[g]
                DtA = dseg(t, 0, A)
                DtV = dseg(t, A, N)
                Dt = dseg(t, 0, N)
                nc.vector.scalar_tensor_tensor(
                    out=DtV, in0=DtV, scalar=st["gamV"][:, i:i + 1],
                    in1=safeb[:, A:N], op0=Al.subtract, op1=Al.mult)
                nc.vector.scalar_tensor_tensor(
                    out=DtA, in0=DtA, scalar=st["gamA"][:, i:i + 1],
                    in1=safeb[:, 0:A], op0=Al.subtract, op1=Al.mult)
                nc.vector.tensor_tensor(Dt, mnb[:], Dt, op=Al.subtract)
                stag = rot.tile([128, N], f32, name=f"rs{t % 2}")
                nc.scalar.copy(stag[:], Dt)
                nc.sync.dma_start(ov[t][:], stag[:])

            # ---- schedule: G0 = tiles 0,1 runs ~3 passes ahead of G1 ----
            startup(0)
            startup(1)
            ginit(0)
            gpass(0, 0)
            startup(2)
            gchain(0)
            gpass(0, 1)
            startup(3)
            safeb = bc_safeb()
            gchain(0)
            gpass(0, 2)
            gchain(0)
            ginit(1)
            for k in range(3, NPASS):
                gpass(1, k - 3)
                gpass(0, k, last=(k == NPASS - 1))
                gchain(1)
                if k < NPASS - 1:
                    gchain(0)
            gfinal_scalars(0)
            gpass(1, 7)
            final(0, safeb)
            gchain(1)
            final(1, safeb)
            gpass(1, 8)
            gchain(1)
            gpass(1, 9, last=True)
            gfinal_scalars(1)
            final(2, safeb)
            final(3, safeb)

    if not nc.is_finalized():
        nc.finalize()
    return nc


def _get_nc():
    if "nc" not in _cache:
        _cache["nc"] = _build()
    return _cache["nc"]


def kernel(x):
    global LAST_RESULT
    from concourse.bass_utils import run_bass_kernel_spmd

    x = np.ascontiguousarray(np.asarray(x), dtype=np.float32)
    a = x.reshape(NCORES * R, N)
    nc = _get_nc()
    in_maps = [{"xs": np.ascontiguousarray(a[c * R:(c + 1) * R])}
               for c in range(NCORES)]
    res = run_bass_kernel_spmd(
        nc, in_maps, core_ids=list(range(NCORES)),
        trace=bool(int(os.environ.get("KBENCH_TRACE", "0"))),
    )
    LAST_RESULT = res
    full = np.concatenate([res.results[c]["out"] for c in range(NCORES)], axis=0)
    return full.reshape(1, NCORES * R, N).astype(np.float32)
